# revision 4
# baseline (speedup 1.0000x reference)
"""Local-strided block-sparse paged attention (decode) on 8 Trainium2 cores.

Strategy (memory-bound problem -> minimize device HBM bytes):
- Host resolves the per-(batch, q-head) CSR rows, then DEDUPLICATES the kv
  blocks across the 4 q-heads of each kv-head group: one gathered K/V panel
  per (b, kv_head) union, with per-head additive masks restoring each head's
  exact row + causal masking.
- Panels are converted to fp16 (halves bytes; ~1e-3 rel err, tolerance 2e-2).
- The 64 panels are assigned to 8 cores x 8 slots sorted by size, so the
  SPMD slot padding (same program on all cores) wastes little bandwidth.
- Device: per panel, stream K [128d, NT] and V [128t, NCH*129] (a ones
  column per 128-token chunk yields the softmax denominator for free in the
  PV matmul), compute all 4 heads per panel in single matmuls:
    QK: stationary K-chunk [128,128], moving q [128,4] -> scores [128, 4]
    PV: stationary p-chunk [128,4], moving V-chunk [128,129] -> out [4,129]
"""
import numpy as np

B, H, KVH, D, X = 16, 16, 4, 128, 4
GRP = H // KVH              # q heads per kv head
BLK, MAXB = 16, 256
NC_CORES = 8
NSLOTS = (B * KVH) // NC_CORES   # panels per core
SM_SCALE = 1.0 / float(np.sqrt(D))
NEG = np.float32(-1e9)

_PROG_CACHE = {}


def _build_device_program(slot_nch):
    import concourse.bacc as bacc
    import concourse.mybir as mybir
    from concourse.tile import TileContext

    f32 = mybir.dt.float32
    f16 = mybir.dt.float16
    nc = bacc.Bacc("TRN2", target_bir_lowering=False)
    kd = [nc.dram_tensor(f"kd{s}", [128, slot_nch[s] * 128], f16, kind="ExternalInput")
          for s in range(NSLOTS)]
    vd = [nc.dram_tensor(f"vd{s}", [128, slot_nch[s] * 129], f16, kind="ExternalInput")
          for s in range(NSLOTS)]
    md = [nc.dram_tensor(f"md{s}", [128, slot_nch[s] * 4], f32, kind="ExternalInput")
          for s in range(NSLOTS)]
    qd = nc.dram_tensor("qd", [128, 4 * NSLOTS], f16, kind="ExternalInput")
    outd = nc.dram_tensor("outd", [NSLOTS, 4, 128], f32, kind="ExternalOutput")

    with TileContext(nc) as tc:
        with (
            tc.tile_pool(name="data", bufs=1) as dp,
            tc.tile_pool(name="work", bufs=3) as wp,
            tc.tile_pool(name="ps_sc", bufs=2, space="PSUM") as psc,
            tc.tile_pool(name="ps_ov", bufs=2, space="PSUM") as pov,
        ):
            qt = dp.tile([128, 4 * NSLOTS], f16, tag="q")
            nc.sync.dma_start(out=qt[:], in_=qd[:])
            kts, vts, mts = [], [], []
            for s in range(NSLOTS):
                kt = dp.tile([128, slot_nch[s] * 128], f16, tag=f"k{s}")
                nc.sync.dma_start(out=kt[:], in_=kd[s][:])
                vt = dp.tile([128, slot_nch[s] * 129], f16, tag=f"v{s}")
                nc.sync.dma_start(out=vt[:], in_=vd[s][:])
                mt = dp.tile([128, slot_nch[s] * 4], f32, tag=f"m{s}")
                nc.sync.dma_start(out=mt[:], in_=md[s][:])
                kts.append(kt); vts.append(vt); mts.append(mt)

            ps = [None] * NSLOTS   # softmax weights per slot (fp16)
            ovs = [None] * NSLOTS  # PV psum accumulators

            def qk_softmax(s):
                nch = slot_nch[s]
                sc = psc.tile([128, nch * 4], f32, tag="sc")
                for c in range(nch):
                    nc.tensor.matmul(
                        sc[:, 4 * c:4 * c + 4],
                        kts[s][:, 128 * c:128 * (c + 1)],
                        qt[:, 4 * s:4 * s + 4],
                        start=True, stop=True,
                    )
                ssb = wp.tile([128, nch * 4], f32, tag="ssb")
                nc.vector.tensor_add(ssb[:], sc[:], mts[s][:])
                p = wp.tile([128, nch * 4], f16, tag="p")
                nc.scalar.activation(
                    p[:], ssb[:], mybir.ActivationFunctionType.Exp,
                    scale=SM_SCALE,
                )
                ps[s] = p

            def pv(s):
                nch = slot_nch[s]
                ov = pov.tile([4, 129], f32, tag="ov")
                for c in range(nch):
                    nc.tensor.matmul(
                        ov[:],
                        ps[s][:, 4 * c:4 * c + 4],
                        vts[s][:, 129 * c:129 * (c + 1)],
                        start=(c == 0), stop=(c == nch - 1),
                    )
                rec = wp.tile([4, 1], f32, tag="rec")
                nc.vector.reciprocal(rec[:], ov[:, 128:129])
                osb = wp.tile([4, 128], f32, tag="osb")
                nc.vector.tensor_scalar_mul(osb[:], ov[:, 0:128], rec[:])
                nc.sync.dma_start(out=outd[s], in_=osb[:])
                ps[s] = None

            # software pipeline: PE does QK(s+1) while ACT produces p(s),
            # then PV(s); PE never waits on the softmax chain.
            qk_softmax(0)
            for s in range(1, NSLOTS):
                qk_softmax(s)
                pv(s - 1)
            pv(NSLOTS - 1)
    nc.compile()
    return nc


def _prep(q, k_cache, v_cache, block_tables, context_lens, layout_crow, layout_col):
    """Resolve CSR rows, dedup kv blocks per (b, kv-head), build panels."""
    q_pid = context_lens.astype(np.int64) - 1            # [B]
    pbid = q_pid // BLK
    h_idx = np.arange(H)
    start = layout_crow[h_idx[None, :], pbid[:, None]]   # [B,H]
    end = layout_crow[h_idx[None, :], pbid[:, None] + 1]

    panels = []  # (nch, b, kv, U[np], cols_per_head[list of 4 arrays])
    for b in range(B):
        for kv in range(KVH):
            cols_h = []
            for dh in range(GRP):
                h = kv * GRP + dh
                cols_h.append(layout_col[h, start[b, h]:end[b, h]])
            U = np.unique(np.concatenate(cols_h))
            nch = max(1, -(-(len(U) * BLK) // 128))
            panels.append((nch, b, kv, U, cols_h))

    order = sorted(range(len(panels)), key=lambda i: -panels[i][0])
    slot_nch = []
    assign = [[None] * NSLOTS for _ in range(NC_CORES)]   # (core,slot) -> panel
    for rank, pi in enumerate(order):
        core, s = rank % NC_CORES, rank // NC_CORES
        assign[core][s] = pi
        if core == 0:
            slot_nch.append(panels[pi][0])
    slot_nch = tuple(slot_nch)

    in_maps = []
    meta = []    # per core: list of (b, kv) per slot
    tok16 = np.arange(BLK)
    for core in range(NC_CORES):
        im = {}
        mt_core = []
        qd = np.zeros((128, 4 * NSLOTS), np.float16)
        for s in range(NSLOTS):
            nch, b, kv, U, cols_h = panels[assign[core][s]]
            NT = slot_nch[s] * 128
            NU = len(U)
            phys = block_tables[b, U]
            kb = k_cache[phys, kv]                       # [NU, 32, 16, 4]
            kt = np.zeros((128, NT), np.float16)
            kt[:, :NU * BLK] = kb.transpose(1, 3, 0, 2).reshape(128, NU * BLK)
            vb = v_cache[phys, kv]                       # [NU, 128, 16]
            v_t = np.zeros((NT, 128), np.float16)
            v_t[:NU * BLK] = vb.transpose(0, 2, 1).reshape(NU * BLK, 128)
            vt = np.ones((slot_nch[s], 128, 129), np.float16)
            vt[:, :, :128] = v_t.reshape(slot_nch[s], 128, 128)
            vt = vt.transpose(1, 0, 2).reshape(128, slot_nch[s] * 129)

            madd = np.full((4, NT), NEG, np.float32)
            upos = U * BLK                               # [NU]
            causal = (upos[:, None] + tok16[None, :]) <= q_pid[b]   # [NU,16]
            for dh in range(GRP):
                allowed = np.isin(U, cols_h[dh])[:, None] & causal
                madd[dh, :NU * BLK] = np.where(allowed.reshape(-1), 0.0, NEG)
            mt = madd.reshape(4, slot_nch[s], 128).transpose(2, 1, 0).reshape(128, slot_nch[s] * 4)

            im[f"kd{s}"] = np.ascontiguousarray(kt)
            im[f"vd{s}"] = np.ascontiguousarray(vt)
            im[f"md{s}"] = np.ascontiguousarray(mt)
            qd[:, 4 * s:4 * s + 4] = q[b, kv * GRP:(kv + 1) * GRP].T
            mt_core.append((b, kv))
        im["qd"] = qd
        in_maps.append(im)
        meta.append(mt_core)
    return slot_nch, in_maps, meta


def kernel(q, k_cache, v_cache, block_tables, context_lens, layout_crow, layout_col):
    import os
    from concourse.bass_utils import run_bass_kernel_spmd

    q = np.asarray(q, np.float32)
    k_cache = np.asarray(k_cache, np.float32)
    v_cache = np.asarray(v_cache, np.float32)
    block_tables = np.asarray(block_tables, np.int32)
    context_lens = np.asarray(context_lens, np.int32)
    layout_crow = np.asarray(layout_crow, np.int32)
    layout_col = np.asarray(layout_col, np.int32)

    slot_nch, in_maps, meta = _prep(
        q, k_cache, v_cache, block_tables, context_lens, layout_crow, layout_col)

    nc = _PROG_CACHE.get(slot_nch)
    if nc is None:
        nc = _build_device_program(slot_nch)
        _PROG_CACHE[slot_nch] = nc

    res = run_bass_kernel_spmd(
        nc, in_maps, core_ids=list(range(NC_CORES)),
        trace=bool(os.environ.get("KERNEL_TRACE")),
    )
    global _LAST_RESULT
    _LAST_RESULT = res

    out = np.empty((B, H, D), np.float32)
    for core in range(NC_CORES):
        for s in range(NSLOTS):
            b, kv = meta[core][s]
            out[b, kv * GRP:(kv + 1) * GRP] = res.results[core]["outd"][s]
    return out


_LAST_RESULT = None


# revision 6
# speedup vs baseline: 1.1658x; 1.1658x over previous
"""Local-strided block-sparse paged attention (decode) on 8 Trainium2 cores.

Strategy (memory-bound -> minimize device HBM bytes and DMA/descriptor serialization):
- Host resolves the per-(batch, q-head) CSR rows, then DEDUPLICATES the kv
  blocks across the 4 q-heads of each kv-head group: one gathered K/V panel
  per (b, kv_head) union, with per-head multiplicative masks restoring each
  head's exact row + causal masking.
- Panels are fp16 (halves bytes; ~5e-4 rel err, tolerance 2e-2).
- The 64 panels are assigned to 8 cores x 8 slots sorted by size, so the
  SPMD slot padding (same program on all cores) wastes little bandwidth.
- Two DMAs per slot: [K|mask] issued by the Sync engine, [V] by the Scalar
  engine -- descriptor generation (~630ns/DMA) runs on both engines in
  parallel instead of serializing on Sync.
- Device per slot (all 4 heads batched per matmul):
    QK:   per 128-token chunk: stationary K-chunk [128d,128t] (FWL),
          moving q [128,4] -> scores[t, 4h] in PSUM
    p   = exp(scores*scale) (ACT, fp16 out) * mask (DVE fp16)
    PV:   per chunk: stationary V-chunk [128t,128d] (FWL), moving p-chunk
          [128,4] -> accumulates out [128d, 4h] in PSUM
    den:  ones[128,1]^T @ p -> per-(chunk,head) sums; host reduces + divides
- Outputs batched into 2 final DMAs; host does the final normalization.
"""
import numpy as np

B, H, KVH, D, X = 16, 16, 4, 128, 4
GRP = H // KVH              # q heads per kv head
BLK, MAXB = 16, 256
NC_CORES = 8
NSLOTS = (B * KVH) // NC_CORES   # panels per core
SM_SCALE = 1.0 / float(np.sqrt(D))

_PROG_CACHE = {}


def _build_device_program(slot_nch):
    import concourse.bacc as bacc
    import concourse.mybir as mybir
    from concourse.tile import TileContext

    f32 = mybir.dt.float32
    f16 = mybir.dt.float16
    nc = bacc.Bacc("TRN2", target_bir_lowering=False)
    # per-slot [K | mask] and [V] panels
    km = [nc.dram_tensor(f"km{s}", [128, slot_nch[s] * 132], f16, kind="ExternalInput")
          for s in range(NSLOTS)]
    vv = [nc.dram_tensor(f"vv{s}", [128, slot_nch[s] * 128], f16, kind="ExternalInput")
          for s in range(NSLOTS)]
    qd = nc.dram_tensor("qd", [128, 4 * NSLOTS], f16, kind="ExternalInput")
    oud = nc.dram_tensor("oud", [128, 4 * NSLOTS], f32, kind="ExternalOutput")
    SUMW = sum(nch * 4 for nch in slot_nch)
    sud = nc.dram_tensor("sud", [1, SUMW], f32, kind="ExternalOutput")

    with TileContext(nc) as tc:
        with (
            tc.tile_pool(name="data", bufs=1) as dp,
            tc.tile_pool(name="work", bufs=3) as wp,
            tc.tile_pool(name="ps_sc", bufs=2, space="PSUM") as psc,
            tc.tile_pool(name="ps_ov", bufs=2, space="PSUM") as pov,
            tc.tile_pool(name="ps_ds", bufs=2, space="PSUM") as pds,
        ):
            qt = dp.tile([128, 4 * NSLOTS], f16, tag="q")
            nc.sync.dma_start(out=qt[:], in_=qd[:])
            ones = dp.tile([128, 1], f16, tag="ones")
            nc.vector.memset(ones[:], 1.0)
            osb = dp.tile([128, 4 * NSLOTS], f32, tag="osb")
            ssb = dp.tile([1, SUMW], f32, tag="ssb")

            kms, vvs = [], []
            for s in range(NSLOTS):
                kmt = dp.tile([128, slot_nch[s] * 132], f16, tag=f"km{s}")
                nc.sync.dma_start(out=kmt[:], in_=km[s][:])
                vvt = dp.tile([128, slot_nch[s] * 128], f16, tag=f"vv{s}")
                nc.scalar.dma_start(out=vvt[:], in_=vv[s][:])
                kms.append(kmt); vvs.append(vvt)

            ps = [None] * NSLOTS
            sum_off = [0] * NSLOTS
            off = 0
            for s in range(NSLOTS):
                sum_off[s] = off
                off += slot_nch[s] * 4

            def qk_softmax(s):
                nch = slot_nch[s]
                sc = psc.tile([128, nch * 4], f32, tag="sc")
                for c in range(nch):
                    nc.tensor.matmul(
                        sc[:, 4 * c:4 * c + 4],
                        kms[s][:, 128 * c:128 * (c + 1)],
                        qt[:, 4 * s:4 * s + 4],
                        start=True, stop=True,
                    )
                p0 = wp.tile([128, nch * 4], f16, tag="p0")
                nc.scalar.activation(
                    p0[:], sc[:], mybir.ActivationFunctionType.Exp,
                    scale=SM_SCALE,
                )
                p = wp.tile([128, nch * 4], f16, tag="p")
                nc.vector.tensor_mul(
                    p[:], p0[:], kms[s][:, nch * 128:nch * 132])
                ps[s] = p

            def pv(s):
                nch = slot_nch[s]
                ov = pov.tile([128, 4], f32, tag="ov")
                for c in range(nch):
                    nc.tensor.matmul(
                        ov[:],
                        vvs[s][:, 128 * c:128 * (c + 1)],
                        ps[s][:, 4 * c:4 * c + 4],
                        start=(c == 0), stop=(c == nch - 1),
                    )
                ds = pds.tile([1, nch * 4], f32, tag="ds")
                nc.tensor.matmul(ds[:], ones[:], ps[s][:], start=True, stop=True)
                nc.vector.tensor_copy(osb[:, 4 * s:4 * s + 4], ov[:])
                nc.vector.tensor_copy(
                    ssb[:, sum_off[s]:sum_off[s] + nch * 4], ds[:])
                ps[s] = None

            # software pipeline: PE does QK(s+1) while ACT/DVE produce p(s);
            # PE never waits on the softmax chain.
            qk_softmax(0)
            for s in range(1, NSLOTS):
                qk_softmax(s)
                pv(s - 1)
            pv(NSLOTS - 1)

            nc.sync.dma_start(out=oud[:], in_=osb[:])
            nc.scalar.dma_start(out=sud[:], in_=ssb[:])
    nc.compile()
    return nc


def _prep(q, k_cache, v_cache, block_tables, context_lens, layout_crow, layout_col):
    """Resolve CSR rows, dedup kv blocks per (b, kv-head), build panels."""
    q_pid = context_lens.astype(np.int64) - 1            # [B]
    pbid = q_pid // BLK
    h_idx = np.arange(H)
    start = layout_crow[h_idx[None, :], pbid[:, None]]   # [B,H]
    end = layout_crow[h_idx[None, :], pbid[:, None] + 1]

    panels = []  # (nch, b, kv, U, cols_per_head)
    for b in range(B):
        for kv in range(KVH):
            cols_h = []
            for dh in range(GRP):
                h = kv * GRP + dh
                cols_h.append(layout_col[h, start[b, h]:end[b, h]])
            U = np.unique(np.concatenate(cols_h))
            nch = max(1, -(-(len(U) * BLK) // 128))
            panels.append((nch, b, kv, U, cols_h))

    order = sorted(range(len(panels)), key=lambda i: -panels[i][0])
    slot_nch = []
    assign = [[None] * NSLOTS for _ in range(NC_CORES)]
    for rank, pi in enumerate(order):
        core, s = rank % NC_CORES, rank // NC_CORES
        assign[core][s] = pi
        if core == 0:
            slot_nch.append(panels[pi][0])
    slot_nch = tuple(slot_nch)

    in_maps = []
    meta = []    # per core: list of (b, kv) per slot
    tok16 = np.arange(BLK)
    for core in range(NC_CORES):
        im = {}
        mt_core = []
        qd = np.zeros((128, 4 * NSLOTS), np.float16)
        for s in range(NSLOTS):
            nch, b, kv, U, cols_h = panels[assign[core][s]]
            NT = slot_nch[s] * 128
            NU = len(U)
            phys = block_tables[b, U]

            kmt = np.zeros((128, slot_nch[s] * 132), np.float16)
            kb = k_cache[phys, kv]                       # [NU, 32, 16, 4]
            kmt[:, :NU * BLK] = kb.transpose(1, 3, 0, 2).reshape(128, NU * BLK)

            vb = v_cache[phys, kv]                       # [NU, 128, 16]
            v_t = np.zeros((NT, 128), np.float16)
            v_t[:NU * BLK] = vb.transpose(0, 2, 1).reshape(NU * BLK, 128)
            vvt = np.ascontiguousarray(
                v_t.reshape(slot_nch[s], 128, 128).transpose(1, 0, 2)
                .reshape(128, NT))

            mm = np.zeros((4, NT), np.float16)
            upos = U * BLK
            causal = (upos[:, None] + tok16[None, :]) <= q_pid[b]   # [NU,16]
            for dh in range(GRP):
                allowed = np.isin(U, cols_h[dh])[:, None] & causal
                mm[dh, :NU * BLK] = allowed.reshape(-1).astype(np.float16)
            kmt[:, NT:] = (
                mm.reshape(4, slot_nch[s], 128).transpose(2, 1, 0)
                .reshape(128, slot_nch[s] * 4))

            im[f"km{s}"] = kmt
            im[f"vv{s}"] = vvt
            qd[:, 4 * s:4 * s + 4] = q[b, kv * GRP:(kv + 1) * GRP].T
            mt_core.append((b, kv))
        im["qd"] = qd
        in_maps.append(im)
        meta.append(mt_core)
    return slot_nch, in_maps, meta


def kernel(q, k_cache, v_cache, block_tables, context_lens, layout_crow, layout_col):
    import os
    from concourse.bass_utils import run_bass_kernel_spmd

    q = np.asarray(q, np.float32)
    k_cache = np.asarray(k_cache, np.float32)
    v_cache = np.asarray(v_cache, np.float32)
    block_tables = np.asarray(block_tables, np.int32)
    context_lens = np.asarray(context_lens, np.int32)
    layout_crow = np.asarray(layout_crow, np.int32)
    layout_col = np.asarray(layout_col, np.int32)

    slot_nch, in_maps, meta = _prep(
        q, k_cache, v_cache, block_tables, context_lens, layout_crow, layout_col)

    nc = _PROG_CACHE.get(slot_nch)
    if nc is None:
        nc = _build_device_program(slot_nch)
        _PROG_CACHE[slot_nch] = nc

    res = run_bass_kernel_spmd(
        nc, in_maps, core_ids=list(range(NC_CORES)),
        trace=bool(os.environ.get("KERNEL_TRACE")),
    )
    global _LAST_RESULT
    _LAST_RESULT = res

    out = np.empty((B, H, D), np.float32)
    for core in range(NC_CORES):
        oud = res.results[core]["oud"]                   # [128, 4*NSLOTS]
        sud = res.results[core]["sud"][0]                # [SUMW]
        off = 0
        for s in range(NSLOTS):
            nch = slot_nch[s]
            b, kv = meta[core][s]
            den = sud[off:off + nch * 4].reshape(nch, 4).sum(0)   # [4]
            out[b, kv * GRP:(kv + 1) * GRP] = (oud[:, 4 * s:4 * s + 4] / den).T
            off += nch * 4
    return out


_LAST_RESULT = None
